# revision 1
# baseline (speedup 1.0000x reference)
"""CAM-GAT layer kernel for 8 Trainium2 NeuronCores (Bass/Tile).

Reference math (per graph of N=21 joints, F=128 feats):
    h = x @ W1                         [N, F]
    s = h @ a1 ; t = h @ a2            [N]
    e[i,j] = leaky_relu(s_i + t_j, 0.2)
    beta = softmax_j(e)
    alpha = cam * beta
    x_agg = alpha @ h
    out = elu(concat([x_agg, x], -1) @ W2_w + W2_b)

Sharding: pure data parallelism over graphs; each of the 8 cores gets
B/8 = 2048 graphs (43008 rows); weights replicated.

Per-core dataflow (supertile = 24 graphs = 504 rows = 4 chunks x 126 rows;
a chunk = 6 graphs of 21 rows on 126 partitions):
  xT    : PE transpose of x rows (bf16)
  h,s   : matmul(lhsT=xT_c, rhs=[W1 | W1@a1]) -> row-major h + s column
  t     : "spread" matmuls -> PSUM [4, 504]: row c holds t of chunk c in
          column block c, zero elsewhere
  e     : one matmul e[i,(c,j)] = t^c_j + MNEG*(1 - sameblock(i,j)) with a
          static stationary; s_i added as the per-partition ACT bias of the
          leaky-relu pass
  smax  : ACT exp (+per-chunk accum) -> DVE reciprocal
  alpha : DVE (E * rinv) * camBD   (block-diag cam zeroes off-blocks)
  aggT  : PE transpose alpha -> alphaT; matmul(lhsT=h_c, rhs=alphaT_c)
  out   : matmul(lhsT=xaggT_c, rhs=W2a) + matmul(lhsT=xT_c, rhs=W2b)
  elu   : z = psum + bias; elu(z) = min(max(z, exp(z)-1), max(z, 0))
"""

import sys

import numpy as np

try:
    import concourse  # noqa: F401
except ImportError:  # pragma: no cover
    sys.path.insert(0, "/opt/trn_rl_repo")

import ml_dtypes
import concourse.bass as bass
import concourse.bacc as bacc
import concourse.tile as tile
from concourse import mybir

FP32 = mybir.dt.float32
BF16 = mybir.dt.bfloat16
AF = mybir.ActivationFunctionType
ALU = mybir.AluOpType

N_JOINTS = 21
F = 128
B_TOTAL = 16384
N_CORES = 8
B_CORE = B_TOTAL // N_CORES            # 2048 graphs per core
ROWS_CORE = B_CORE * N_JOINTS          # 43008 rows per core

G_CHUNK = 6                            # graphs per chunk
RC = G_CHUNK * N_JOINTS                # 126 rows per chunk
NCH = 4                                # chunks per (full) supertile
ROWS_SUPER = NCH * RC                  # 504
MNEG = -60000.0                        # exp(0.2*MNEG) == 0 in fp32


def _row_plan(rows):
    plan = []
    r = 0
    while r < rows:
        st = min(ROWS_SUPER, rows - r)
        assert st % N_JOINTS == 0
        chunks = []
        c = 0
        while c < st:
            chunks.append(min(RC, st - c))
            c += RC
        plan.append(chunks)
        r += st
    return plan


def host_consts(cam, W1, a, W2_w, W2_b):
    """Precompute tiny replicated tensors on the host (numpy)."""
    cam = np.asarray(cam, np.float32)
    W1 = np.asarray(W1, np.float32)
    a = np.asarray(a, np.float32)
    W2_w = np.asarray(W2_w, np.float32)
    W2_b = np.asarray(W2_b, np.float32)
    wa1 = W1 @ a[:F]                   # [128]
    wa2 = W1 @ a[F:]                   # [128]

    # waS[c]/waT[c]: [128, 4], col c = wa1 / wa2 (s-dense / t-spread mms)
    waS = np.zeros((NCH, F, NCH), np.float32)
    waT = np.zeros((NCH, F, NCH), np.float32)
    for c in range(NCH):
        waS[c, :, c] = wa1
        waT[c, :, c] = wa2

    blk = np.arange(RC) // N_JOINTS
    # e-matmul operands, contraction k in [0, 36):
    #   k 0-3  : L dynamic s rows      | R static delta(c==c') ones
    #   k 4-9  : L MNEG*(blk(i)==q)    | R (1 - (blk(j)==q)) tiled
    #   k 10-31: zeros (partition-alignment filler)
    #   k 32-35: L ones                | R dynamic t-spread rows
    L36 = np.zeros((36, RC), np.float32)
    for q in range(G_CHUNK):
        L36[4 + q, :] = MNEG * (blk == q)
    L36[32:36, :] = 1.0

    R36 = np.zeros((36, ROWS_SUPER), np.float32)
    for c in range(NCH):
        R36[c, c * RC:(c + 1) * RC] = 1.0
    for q in range(G_CHUNK):
        pat = (blk != q).astype(np.float32)
        for c in range(NCH):
            R36[4 + q, c * RC:(c + 1) * RC] = pat

    # Block-diag cam replicated: camBD[i, c*126+j] = cam[i%21, j%21] if
    # i//21 == j//21 else 0
    camblk = np.zeros((RC, RC), np.float32)
    for q in range(G_CHUNK):
        camblk[q * N_JOINTS:(q + 1) * N_JOINTS,
               q * N_JOINTS:(q + 1) * N_JOINTS] = cam
    camBD = np.tile(camblk, (1, NCH))   # [126, 504]

    bf = ml_dtypes.bfloat16
    return {
        "w1b": W1.astype(bf),                        # [128,128]
        "w2ab": W2_w[:F].astype(bf),                 # [128,128]
        "w2bb": W2_w[F:].astype(bf),                 # [128,128]
        "was_": waS.astype(bf),                      # [4,128,4]
        "wat": waT.astype(bf),                       # [4,128,4]
        "l36": L36.astype(bf),                       # [36,126]
        "r36": R36.astype(bf),                       # [36,504]
        "cambd": camBD.astype(np.float32),           # [126,504]
        "biasr": np.tile(W2_b, (RC, 1)).astype(np.float32),  # [126,128]
        "ident": np.eye(RC, dtype=bf),               # [126,126]
    }


CONST_SPECS = {
    "w1b": ([F, F], BF16),
    "w2ab": ([F, F], BF16),
    "w2bb": ([F, F], BF16),
    "was_": ([NCH, F, NCH], BF16),
    "wat": ([NCH, F, NCH], BF16),
    "l36": ([36, RC], BF16),
    "r36": ([36, ROWS_SUPER], BF16),
    "cambd": ([RC, ROWS_SUPER], FP32),
    "biasr": ([RC, F], FP32),
    "ident": ([RC, RC], BF16),
}


def build_program(rows=ROWS_CORE):
    nc = bacc.Bacc("TRN2", target_bir_lowering=False, debug=False,
                   enable_asserts=False)
    x_d = nc.dram_tensor("x", [rows, F], FP32, kind="ExternalInput").ap()
    out_d = nc.dram_tensor("out", [rows, F], FP32, kind="ExternalOutput").ap()
    cst = {k: nc.dram_tensor(k, shape, dt, kind="ExternalInput").ap()
           for k, (shape, dt) in CONST_SPECS.items()}
    with tile.TileContext(nc) as tc:
        _body(tc, x_d, out_d, cst, rows)
    nc.compile()
    return nc


def _bcast_c(ap, n):
    """Insert a stride-0 dim after the partition dim: [P, X] -> [P, n, X]."""
    p, rest = ap.ap[0], list(ap.ap[1:])
    assert len(rest) == 1
    return bass.AP(ap.tensor, ap.offset, [p, [0, n], rest[0]])


def _body(tc, x_d, out_d, cst, rows):
    from contextlib import ExitStack
    nc = tc.nc
    plan = _row_plan(rows)

    with ExitStack() as ctx:
        # ---- pools ----
        cpool = ctx.enter_context(tc.tile_pool(name="consts", bufs=1))
        pin = ctx.enter_context(tc.tile_pool(name="xin", bufs=3))
        pxt = ctx.enter_context(tc.tile_pool(name="xt", bufs=2))
        ph = ctx.enter_context(tc.tile_pool(name="h", bufs=2))
        psb = ctx.enter_context(tc.tile_pool(name="ssb", bufs=2))
        plr = ctx.enter_context(tc.tile_pool(name="lr", bufs=2))
        pe_ = ctx.enter_context(tc.tile_pool(name="esb", bufs=2))
        psc = ctx.enter_context(tc.tile_pool(name="scal", bufs=2))
        pa = ctx.enter_context(tc.tile_pool(name="alpha", bufs=2))
        pat = ctx.enter_context(tc.tile_pool(name="alphat", bufs=2))
        pxa = ctx.enter_context(tc.tile_pool(name="xagg", bufs=2))
        pz = ctx.enter_context(tc.tile_pool(name="zbuf", bufs=2))
        pem = ctx.enter_context(tc.tile_pool(name="embuf", bufs=2))
        pp1 = ctx.enter_context(tc.tile_pool(name="p1buf", bufs=2))
        pout = ctx.enter_context(tc.tile_pool(name="outsb", bufs=3))

        ps_e = ctx.enter_context(tc.tile_pool(name="ps_e", bufs=1, space="PSUM"))
        ps_s = ctx.enter_context(tc.tile_pool(name="ps_s", bufs=1, space="PSUM"))
        ps_t = ctx.enter_context(tc.tile_pool(name="ps_t", bufs=1, space="PSUM"))
        ps_xt = ctx.enter_context(tc.tile_pool(name="ps_xt", bufs=1, space="PSUM"))
        ps_h = ctx.enter_context(tc.tile_pool(name="ps_h", bufs=1, space="PSUM"))
        ps_at = ctx.enter_context(tc.tile_pool(name="ps_at", bufs=1, space="PSUM"))
        ps_xa = ctx.enter_context(tc.tile_pool(name="ps_xa", bufs=1, space="PSUM"))
        ps_o = ctx.enter_context(tc.tile_pool(name="ps_o", bufs=1, space="PSUM"))

        # ---- load constants ----
        w1b = cpool.tile([F, F], BF16, tag="w1b")
        w2ab = cpool.tile([F, F], BF16, tag="w2ab")
        w2bb = cpool.tile([F, F], BF16, tag="w2bb")
        was_ = cpool.tile([F, NCH, NCH], BF16, tag="was_")
        wat = cpool.tile([F, NCH, NCH], BF16, tag="wat")
        cambd = cpool.tile([RC, ROWS_SUPER], FP32, tag="cambd")
        biasr = cpool.tile([RC, F], FP32, tag="biasr")
        ident = cpool.tile([RC, RC], BF16, tag="ident")
        nc.sync.dma_start(w1b[:], cst["w1b"][:])
        nc.sync.dma_start(w2ab[:], cst["w2ab"][:])
        nc.sync.dma_start(w2bb[:], cst["w2bb"][:])
        nc.sync.dma_start(was_[:], cst["was_"].rearrange("c f e -> f c e"))
        nc.sync.dma_start(wat[:], cst["wat"].rearrange("c f e -> f c e"))
        nc.sync.dma_start(cambd[:], cst["cambd"][:])
        nc.sync.dma_start(biasr[:], cst["biasr"][:])
        nc.sync.dma_start(ident[:], cst["ident"][:])

        # L/R e-matmul tiles (even/odd persistent): dynamic rows 0-3 (s) on
        # L and 32-35 (t-spread) on R; everything else static
        LRs = []
        for par in ("ev", "od"):
            Lt = cpool.tile([36, RC], BF16, tag=f"L_{par}")
            Rt = cpool.tile([36, ROWS_SUPER], BF16, tag=f"R_{par}")
            nc.sync.dma_start(Lt[:], cst["l36"][:])
            nc.sync.dma_start(Rt[:], cst["r36"][:])
            LRs.append((Lt, Rt))

        r0 = 0
        for sti, chunks in enumerate(plan):
            nch = len(chunks)
            st_rows = sum(chunks)
            Lt, Rt = LRs[sti % 2]

            # -- load x (f32 -> bf16 cast DMA on gpsimd) --
            x_bf = pin.tile([RC, NCH, F], BF16, tag="x_bf")
            for c in range(nch):
                rc = chunks[c]
                if rc < RC:
                    # partial chunk: zero so junk never reaches the s/t mms
                    nc.gpsimd.memset(x_bf[:, c, :], 0.0)
                nc.gpsimd.dma_start(
                    x_bf[0:rc, c, :],
                    x_d[r0 + c * RC:r0 + c * RC + rc, :])

            # -- transpose x chunks: xT[f, c, j] --
            xt_ps = ps_xt.tile([F, NCH, RC], BF16, tag="xt_ps")
            for c in range(nch):
                nc.tensor.transpose(xt_ps[:, c, :], x_bf[:, c, :], ident[:])
            xt = pxt.tile([F, NCH, RC], BF16, tag="xt")
            nc.vector.tensor_copy(xt[:, :nch, :], xt_ps[:, :nch, :])

            # -- h row-major: lhsT = xT_c, rhs = W1 --
            h_ps = ps_h.tile([RC, NCH, F], FP32, tag="h_ps")
            for c in range(nch):
                nc.tensor.matmul(h_ps[:, c, :], xt[:, c, :], w1b[:],
                                 start=True, stop=True)
            h = ph.tile([RC, NCH, F], BF16, tag="h")
            nc.vector.tensor_copy(h[:, :nch, :], h_ps[:, :nch, :])

            # -- s dense: accumulate so row c = s of chunk c --
            s_ps = ps_s.tile([NCH, RC], FP32, tag="s_ps")
            for c in range(nch):
                nc.tensor.matmul(s_ps[:], was_[:, c, :], xt[:, c, :],
                                 start=(c == 0), stop=(c == nch - 1))
            nc.vector.tensor_copy(Lt[0:4, :], s_ps[:])

            # -- t spread: row c = t of chunk c in column block c --
            t_ps = ps_t.tile([NCH, ROWS_SUPER], FP32, tag="t_ps")
            for c in range(nch):
                rc = chunks[c]
                nc.tensor.matmul(t_ps[:, c * RC:c * RC + rc],
                                 wat[:, c, :], xt[:, c, 0:rc],
                                 start=True, stop=True)
            nc.vector.tensor_copy(Rt[32:36, 0:st_rows], t_ps[:, 0:st_rows])

            # -- e = s + t + mask (one matmul) --
            e_ps = ps_e.tile([RC, ROWS_SUPER], FP32, tag="e_ps")
            nc.tensor.matmul(e_ps[:, 0:st_rows], Lt[:], Rt[:, 0:st_rows],
                             start=True, stop=True)

            # -- softmax: lrelu via 0.8*(0.25*e + relu(e)); exp w/ scale --
            rel = plr.tile([RC, ROWS_SUPER], FP32, tag="rel")
            nc.scalar.activation(rel[:, 0:st_rows], e_ps[:, 0:st_rows],
                                 AF.Relu)
            u = plr.tile([RC, ROWS_SUPER], FP32, tag="u")
            nc.vector.scalar_tensor_tensor(
                u[:, 0:st_rows], e_ps[:, 0:st_rows], 0.25, rel[:, 0:st_rows],
                op0=ALU.mult, op1=ALU.add)
            E = pe_.tile([RC, ROWS_SUPER], FP32, tag="E")
            rowsum = psc.tile([RC, NCH], FP32, tag="rowsum")
            for c in range(nch):
                rc = chunks[c]
                sl = slice(c * RC, c * RC + rc)
                nc.scalar.activation(E[:, sl], u[:, sl], AF.Exp, scale=0.8,
                                     accum_out=rowsum[:, c:c + 1])
            rinv = psc.tile([RC, NCH], FP32, tag="rinv")
            nc.vector.reciprocal(rinv[:, 0:nch], rowsum[:, 0:nch])

            # -- alpha = (E * rinv) * camBD  (bf16) --
            A = pa.tile([RC, ROWS_SUPER], BF16, tag="A")
            for c in range(nch):
                rc = chunks[c]
                sl = slice(c * RC, c * RC + rc)
                nc.vector.scalar_tensor_tensor(
                    A[:, sl], E[:, sl], rinv[:, c:c + 1], cambd[:, sl],
                    op0=ALU.mult, op1=ALU.mult)

            # -- alphaT via PE transpose --
            at_ps = ps_at.tile([RC, NCH, RC], BF16, tag="at_ps")
            for c in range(nch):
                rc = chunks[c]
                nc.tensor.transpose(at_ps[0:rc, c, :],
                                    A[:, c * RC:c * RC + rc], ident[:])
            at = pat.tile([RC, NCH, RC], BF16, tag="at")
            for c in range(nch):
                rc = chunks[c]
                nc.vector.tensor_copy(at[0:rc, c, :], at_ps[0:rc, c, :])

            # -- x_aggT: lhsT = h_c, rhs = alphaT_c --
            xa_ps = ps_xa.tile([F, NCH, RC], FP32, tag="xa_ps")
            for c in range(nch):
                rc = chunks[c]
                nc.tensor.matmul(xa_ps[:, c, :], h[0:rc, c, :],
                                 at[0:rc, c, :], start=True, stop=True)
            xa = pxa.tile([F, NCH, RC], BF16, tag="xa")
            nc.vector.tensor_copy(xa[:, :nch, :], xa_ps[:, :nch, :])

            # -- out_c = x_aggT_c.T @ W2a + xT_c.T @ W2b --
            o_ps = ps_o.tile([RC, NCH, F], FP32, tag="o_ps")
            for c in range(nch):
                nc.tensor.matmul(o_ps[:, c, :], xa[:, c, :], w2ab[:],
                                 start=True, stop=False)
                nc.tensor.matmul(o_ps[:, c, :], xt[:, c, :], w2bb[:],
                                 start=False, stop=True)

            # -- epilogue: z = psum + bias; elu(z)=min(max(z,e^z-1),max(z,0))
            z = pz.tile([RC, NCH, F], FP32, tag="z")
            nc.vector.tensor_tensor(
                z[:, :nch, :], o_ps[:, :nch, :],
                _bcast_c(biasr[:], nch), ALU.add)
            em = pem.tile([RC, NCH, F], FP32, tag="em")
            nc.scalar.activation(em[:, :nch, :], z[:, :nch, :], AF.Exp)
            p1 = pp1.tile([RC, NCH, F], FP32, tag="p1")
            nc.vector.scalar_tensor_tensor(
                p1[:, :nch, :], em[:, :nch, :], -1.0, z[:, :nch, :],
                op0=ALU.add, op1=ALU.max)
            ot = pout.tile([RC, NCH, F], FP32, tag="ot")
            nc.vector.scalar_tensor_tensor(
                ot[:, :nch, :], z[:, :nch, :], 0.0, p1[:, :nch, :],
                op0=ALU.max, op1=ALU.min)

            # -- store --
            for c in range(nch):
                rc = chunks[c]
                nc.sync.dma_start(
                    out_d[r0 + c * RC:r0 + c * RC + rc, :],
                    ot[0:rc, c, :])
            r0 += st_rows


# ---------------------------------------------------------------------------
_PROG_CACHE = {}


def _get_program(rows):
    if rows not in _PROG_CACHE:
        _PROG_CACHE[rows] = build_program(rows)
    return _PROG_CACHE[rows]


def kernel(x, cam, W1, a, W2_w, W2_b):
    from concourse.bass_utils import run_bass_kernel_spmd

    x = np.ascontiguousarray(np.asarray(x, np.float32))
    consts = host_consts(cam, W1, a, W2_w, W2_b)
    nc = _get_program(ROWS_CORE)

    in_maps = []
    for core in range(N_CORES):
        m = {"x": x[core * ROWS_CORE:(core + 1) * ROWS_CORE]}
        m.update(consts)
        in_maps.append(m)
    res = run_bass_kernel_spmd(nc, in_maps, list(range(N_CORES)))
    out = np.concatenate([res.results[i]["out"] for i in range(N_CORES)], axis=0)
    return out.astype(np.float32)

